# revision 56
# baseline (speedup 1.0000x reference)
# kernel.py — DiscriminativeLoss on 8 TRN2 NeuronCores (Bass/Tile, SPMD).
#
# Math (matches reference):
#   counts_k = #{i: l_i = k};  S_k = sum_{i in k} x_i;  mu_k = S_k / max(c_k, 1)
#   intra = (1/K) * sum_i invc_{l_i} * relu(||x_i - mu_{l_i} + eps|| - 1.5)^2
#   inter = sum_{a != b} relu(1 - ||(mu_a + eps) - mu_b||)^2 / (K*(K-1))
#   reg   = (1/K) * sum_k ||mu_k + eps||
#   total = intra + inter + 0.001 * reg
#
# Device strategy (per core, data-parallel over points; point i = p*tpc + j
# lives at [p, j]):
#   pass 1: one-hot H2 [128, 64, J1] built per chunk via a single DVE
#     tensor_tensor is_equal against a materialized replicated iota (all
#     operands packed 2-byte -> 2x DVE mode); per-tile PE matmuls
#     lhsT=H2[:, :, j] [128, 64] x rhs=xe[:, j, :] accumulate S^T [64, 32]
#     and (x ones-rhs) counts [64, 1] (N<=32 -> cheap).
#   AllReduce [64, 33] across 8 cores (28us fixed cost; overlapped with
#     pass-2 one-hot prebuilds, label-row DMA prefetch, and PE keep-warm
#     dummy matmuls that hold the tensor engine's p-state at full clock).
#   stats: invc = 1/max(c,1), mu = S*invc; the reduced block is DMA'd back
#     duplicated into both partition halves so table [128, 33] =
#     [mu-eps | invc] (rows 64:128 serve the B-half pairing) needs no
#     replication hop.
#   pass 2, per outer chunk of 15 A-tiles + 15 B-tiles: transposed one-hot
#     ht [128, 15*128] built at 4x DVE (TensorScalarPtr is_equal vs the
#     per-partition iota; Pool builds the later chunks) from a
#     broadcast-DMA'd label row; per tile, PE accumulates
#     psum[:, slot, :] = gather(mu-eps) - x  (the diff formed entirely on
#     PE via a -Identity matmul) and a 1-col gather of invc into slots
#     30/31 of the bank-aligned [128, 32, 32] psum tile; Act squares the
#     PSUM diff to bf16; DVE reduces via log2 halving adds (packed bf16 ->
#     2x mode) and extracts invc.
#   finals: dist = sqrt(d2); hinge = relu(dist-1.5); intra partial =
#     sum hinge^2 * invc via muls + row reduce (chained across two column
#     segments) + ones-matmul partition reduce.
#   inter/reg (KxK) replicated on every core from the reduced stats.
#   (GPSIMD constraints honored for real HW: no PSUM access, no is_equal
#   TensorTensor; all one-hot compares on Pool use TensorScalarPtr.)
import math
import numpy as np
from contextlib import ExitStack

import concourse.bass as bass
import concourse.bacc as bacc
import concourse.tile as tile
import concourse.mybir as mybir
from concourse.bass_utils import run_bass_kernel_spmd

F32 = mybir.dt.float32
BF16 = mybir.dt.bfloat16
I16 = mybir.dt.int16

N_CORES = 8
K = 64
D = 32
P = 128
EPS = 1e-8
PAD_LABEL = 512  # never matches any one-hot row (0..127); exact in bf16

INTRA_MARGIN = 1.5
INTER_MARGIN2 = 1.0  # 2 * 0.5

J1 = 60       # pass-1 tiles per one-hot chunk
JMG = 15      # pass-2 A-tiles (and B-tiles) per outer chunk
PREBUILD = 21   # pass-2 ht chunks emitted before the collective section
L2_BUFS = 5
HT_BUFS = 20


def _host_prep(features, labels, tpc):
    """Shard + relayout on host. Returns per-core input dicts."""
    n_total = features.shape[0]
    n_core = n_total // N_CORES
    n_pad = P * tpc
    import ml_dtypes

    na = (tpc + 1) // 2
    nout = math.ceil(na / JMG)
    iotacol = np.arange(P, dtype=np.float32).reshape(P, 1)
    negid = (-np.eye(P)).astype(ml_dtypes.bfloat16)
    id64 = np.eye(K, dtype=np.float32)
    eyeneg = (1.0 - np.eye(K, dtype=np.float32)).astype(ml_dtypes.bfloat16)

    in_maps = []
    for c in range(N_CORES):
        f = np.asarray(features[c * n_core : (c + 1) * n_core], dtype=np.float32)
        l = np.asarray(labels[c * n_core : (c + 1) * n_core], dtype=np.int64)
        if n_pad > n_core:
            f = np.concatenate([f, np.zeros((n_pad - n_core, D), np.float32)], axis=0)
            l = np.concatenate([l, np.full((n_pad - n_core,), PAD_LABEL, np.int64)])
        # xe: [P, tpc, 32] bf16 (counts come from separate ones-rhs matmuls)
        xe = f.reshape(P, tpc, D).astype(ml_dtypes.bfloat16)
        # p-major labels (pass-1 one-hot): [P, tpc] bf16 (values exact)
        lpm = l.reshape(P, tpc).astype(ml_dtypes.bfloat16)
        # tile-major labels for pass 2: ltm [nout, 2, JMG*P] int16,
        # [oc, 0] = A-tile labels, [oc, 1] = B-tile labels + 64.
        ltm_full = l.reshape(P, tpc).T.astype(np.float32)  # [tpc, P]
        ltm = np.full((nout, 2, JMG * P), PAD_LABEL, ml_dtypes.bfloat16)
        for oc in range(nout):
            a0 = oc * JMG
            an = min(JMG, na - a0)
            ltm[oc, 0, : an * P] = ltm_full[a0 : a0 + an].ravel()
            b0 = na + a0
            bn = max(0, min(JMG, tpc - b0))
            if bn > 0:
                ltm[oc, 1, : bn * P] = ltm_full[b0 : b0 + bn].ravel() + 64
        in_maps.append(
            {
                "xe": np.ascontiguousarray(xe),
                "lpm": np.ascontiguousarray(lpm),
                "ltm": np.ascontiguousarray(ltm),
                "iotacol": iotacol,
                "negid": negid,
                "id64": id64,
                "eyeneg": eyeneg,
            }
        )
    return in_maps


def build_program(tpc):
    """Build the SPMD Bass program. tpc = tiles per core (cols per partition)."""
    nc = bacc.Bacc(
        "TRN2", target_bir_lowering=False, debug=False, num_devices=N_CORES
    )
    core_ids = list(range(N_CORES))

    na = (tpc + 1) // 2
    nout = math.ceil(na / JMG)
    n_chunks1 = math.ceil(tpc / J1)

    xe_d = nc.dram_tensor("xe", [P, tpc, D], BF16, kind="ExternalInput").ap()
    lpm_d = nc.dram_tensor("lpm", [P, tpc], BF16, kind="ExternalInput").ap()
    ltm_d = nc.dram_tensor("ltm", [nout, 2, JMG * P], BF16, kind="ExternalInput").ap()
    iotacol_d = nc.dram_tensor("iotacol", [P, 1], F32, kind="ExternalInput").ap()
    negid_d = nc.dram_tensor("negid", [P, P], BF16, kind="ExternalInput").ap()
    id64_d = nc.dram_tensor("id64", [K, K], F32, kind="ExternalInput").ap()
    eyeneg_d = nc.dram_tensor("eyeneg", [K, K], BF16, kind="ExternalInput").ap()
    out_d = nc.dram_tensor("out", [3], F32, kind="ExternalOutput").ap()

    with tile.TileContext(nc, num_cores=N_CORES) as tc, ExitStack() as ctx:
        singles = ctx.enter_context(tc.tile_pool(name="singles", bufs=1))
        xpool = ctx.enter_context(tc.tile_pool(name="xpool", bufs=1))
        hpool = ctx.enter_context(tc.tile_pool(name="hpool", bufs=2))
        l2pool = ctx.enter_context(tc.tile_pool(name="l2pool", bufs=L2_BUFS))
        htpool = ctx.enter_context(tc.tile_pool(name="htpool", bufs=HT_BUFS))
        sqpool = ctx.enter_context(tc.tile_pool(name="sqpool", bufs=3))
        hvpool = ctx.enter_context(tc.tile_pool(name="hvpool", bufs=3))
        wpool = ctx.enter_context(tc.tile_pool(name="wpool", bufs=2))
        psA = ctx.enter_context(tc.tile_pool(name="psA", bufs=1, space="PSUM"))
        psMg = ctx.enter_context(tc.tile_pool(name="psMg", bufs=3, space="PSUM"))
        psS = ctx.enter_context(tc.tile_pool(name="psS", bufs=1, space="PSUM"))
        dram = ctx.enter_context(tc.tile_pool(name="dram", bufs=2, space="DRAM"))

        # ---------- constants (critical first: pass-1 inputs) ----------
        lpm = singles.tile([P, tpc], BF16)
        # first pass-1 chunk's labels land first (tiny strided DMA) so the
        # H2 stream starts ~0.6us earlier; the rest follows immediately
        j1c = min(J1, tpc)
        nc.sync.dma_start(out=lpm[:, :j1c], in_=lpm_d[:, :j1c])
        if tpc > j1c:
            nc.sync.dma_start(out=lpm[:, j1c:], in_=lpm_d[:, j1c:])
        # replicated iota built on-chip by the (otherwise idle) Pool engine:
        # keeps the pass-1 critical path down to the single small lpm DMA
        iota_rep = singles.tile([P, K, J1], BF16)
        nc.gpsimd.iota(
            iota_rep, pattern=[[1, K], [0, J1]], base=0,
            channel_multiplier=0, allow_small_or_imprecise_dtypes=True,
        )
        margneg = singles.tile([P, 1], F32)
        nc.vector.memset(margneg, -float(INTRA_MARGIN))
        ones64 = singles.tile([K, 1], F32)
        nc.vector.memset(ones64, 1.0)
        # prewarm the Act function table with Sqrt: narrows the possible
        # table sets to one containing sqrt+square+relu+copy, so the single
        # 1.3us load happens here, off-critical, and never again
        actwarm = singles.tile([1, 1], F32)
        nc.scalar.activation(
            out=actwarm, in_=ones64[0:1, :],
            func=mybir.ActivationFunctionType.Sqrt,
        )
        xe = xpool.tile([P, tpc, D], BF16)
        ones128 = singles.tile([P, 1], BF16)
        nc.vector.memset(ones128, 1.0)

        d2all = singles.tile([P, tpc], BF16)
        invc_all = singles.tile([P, tpc], BF16)

        # ---------- pass 1: segment sums S^T [64, 32] + counts [64, 1] ----------
        psumS = psA.tile([K, D], F32)
        psumC = psS.tile([K, 1], F32, tag="small")
        t_done = 0
        for c in range(n_chunks1):
            j0 = c * J1
            jn = min(J1, tpc - j0)
            nc.sync.dma_start(
                out=xe[:, j0 : j0 + jn, :], in_=xe_d[:, j0 : j0 + jn, :]
            )
            h2 = hpool.tile([P, K, J1], BF16, tag="h2")
            nc.vector.tensor_tensor(
                h2[:, :, :jn],
                lpm[:, None, j0 : j0 + jn].to_broadcast((P, K, jn)),
                iota_rep[:, :, :jn],
                mybir.AluOpType.is_equal,
            )
            for j in range(jn):
                nc.tensor.matmul(
                    psumS,
                    h2[:, :, j],
                    xe[:, j0 + j, :],
                    start=(t_done == 0),
                    stop=(t_done == tpc - 1),
                )
                nc.tensor.matmul(
                    psumC,
                    h2[:, :, j],
                    ones128,
                    start=(t_done == 0),
                    stop=(t_done == tpc - 1),
                )
                t_done += 1

        # ---------- remaining constants (needed only from pass 2 on) ----------
        iotacol = singles.tile([P, 1], F32)
        nc.sync.dma_start(out=iotacol, in_=iotacol_d)
        negid = singles.tile([P, P], BF16)
        nc.sync.dma_start(out=negid, in_=negid_d)
        id64 = singles.tile([K, K], F32)
        nc.sync.dma_start(out=id64, in_=id64_d)
        eyeneg = singles.tile([K, K], BF16)
        nc.sync.dma_start(out=eyeneg, in_=eyeneg_d)

        # ---------- pass-2 prep: prebuild label rows + transposed one-hots ----
        # (no dependency on the collective -> fills the AllReduce window)
        l2_tiles = {}
        ht_tiles = {}

        def emit_l2_ht(oc, eng=nc.vector):
            src = ltm_d[oc]
            l2 = l2pool.tile([P, JMG * P], BF16, tag="l2")
            nc.sync.dma_start(
                out=l2,
                in_=bass.AP(
                    tensor=src.tensor,
                    offset=src.offset,
                    ap=[[JMG * P, 2], [0, K]] + [[1, JMG * P]],
                ),
            )
            ht = htpool.tile([P, JMG * P], BF16, tag="ht")
            eng.tensor_single_scalar(
                ht, l2, iotacol, mybir.AluOpType.is_equal
            )
            l2_tiles[oc] = l2
            ht_tiles[oc] = ht

        # ---------- early pass-2 one-hot prebuilds (before the stats block
        # so the l2-gated tail can't delay the table computation) ----------
        for oc in range(min(15, nout)):
            emit_l2_ht(oc)

        # ---------- AllReduce the [64, 33] stats ----------
        sg_local = wpool.tile([K, D + 1], F32, tag="sg")
        nc.scalar.copy(out=sg_local[:, :D], in_=psumS)
        nc.scalar.copy(out=sg_local[:, D : D + 1], in_=psumC)
        cc_in = dram.tile([K, D + 1], F32)
        cc_out = dram.tile([K, D + 1], F32)
        nc.gpsimd.dma_start(out=cc_in, in_=sg_local)
        nc.gpsimd.collective_compute(
            "AllReduce",
            mybir.AluOpType.add,
            replica_groups=[core_ids],
            ins=[cc_in.opt()],
            outs=[cc_out.opt()],
        )

        # ---------- PE keep-warm during the collective ----------
        # the tensor engine p-state drops after ~idle; feed it junk matmuls
        # (into the recycled small-psum slot) so pass-2 gathers start at
        # full clock. No data deps; they fill the AllReduce window.
        junkps = psS.tile([K, 512], F32, tag="small")
        warm_tiles = min(16, tpc)
        for w in range(150):
            nc.tensor.matmul(
                junkps[:, : warm_tiles * D], negid[:, :K],
                xe[:, 0:warm_tiles, :], start=True, stop=True,
            )

        # ---------- remaining prebuilds + all Pool-side one-hot builds ------
        # (DVE 16..20 after the stats so they can't delay the table; Pool
        # takes the rest, always ahead of consumption via the ht ring)
        for oc in range(15, nout):
            emit_l2_ht(oc, nc.gpsimd)


        # per-point finals tiles + helper (emitted in segments so most of
        # the sqrt/relu/mul/accumulate work overlaps pass 2). The running
        # per-partition intra sum is chained through tensor_tensor_reduce's
        # accumulator seed.
        dist = singles.tile([P, tpc], BF16)
        hinge = singles.tile([P, tpc], BF16)
        hsq = dist  # dist is dead after the relu; reuse its storage
        hv = singles.tile([P, tpc], BF16)
        seg_done = (0, na)
        racc_tiles = []

        def emit_finals_segment(ca0, ca1, cb0, cb1):
            for c0, c1 in ((ca0, ca1), (cb0, cb1)):
                if c1 <= c0:
                    continue
                nc.scalar.activation(
                    out=dist[:, c0:c1], in_=d2all[:, c0:c1],
                    func=mybir.ActivationFunctionType.Sqrt,
                )
                nc.scalar.activation(
                    out=hinge[:, c0:c1], in_=dist[:, c0:c1],
                    func=mybir.ActivationFunctionType.Relu, bias=margneg,
                )
                nc.vector.tensor_mul(
                    hsq[:, c0:c1], hinge[:, c0:c1], hinge[:, c0:c1]
                )
                racc = singles.tile([P, 1], F32, tag=f"racc{len(racc_tiles)}")
                nc.vector.tensor_mul(hv[:, c0:c1], hsq[:, c0:c1], invc_all[:, c0:c1])
                nc.vector.tensor_reduce(
                    out=racc, in_=hv[:, c0:c1], axis=mybir.AxisListType.X,
                    op=mybir.AluOpType.add,
                )
                if racc_tiles:
                    nc.vector.tensor_add(racc, racc, racc_tiles[-1])
                racc_tiles.append(racc)

        # fetch the reduced stats duplicated into both partition halves so
        # the whole [128, 33] table is computed in one go (no replication
        # hop). high_priority so the scheduler doesn't slot a 1.6us Pool
        # one-hot build ahead of it.
        sg = singles.tile([P, D + 1], F32)
        nc.scalar.dma_start(
            out=sg,
            in_=bass.AP(
                tensor=cc_out.tensor,
                offset=cc_out.offset,
                ap=[[0, 2], [D + 1, K], [1, D + 1]],
            ),
        )

        # ---------- critical stats: table = [mu - eps | invc] ----------
        # high_priority: the scheduler must not queue l2-gated one-hot
        # builds ahead of these on DVE -- the whole of pass 2 waits on table
        with tc.high_priority():
            safec = singles.tile([P, 1], F32)
            nc.vector.tensor_scalar_max(safec, sg[:, D : D + 1], 1.0)
            invc = singles.tile([P, 1], F32)
            nc.vector.reciprocal(invc, safec)
            mu = singles.tile([P, D], F32)
            nc.vector.tensor_mul(mu, sg[:, :D], invc.to_broadcast((P, D)))
            table = singles.tile([P, D + 1], BF16)
            nc.vector.tensor_scalar_add(table[:, :D], mu, -EPS)
            nc.vector.tensor_scalar_add(table[:, D : D + 1], invc, 0.0)

        # ---------- pass 2: diff via PE, square on Act, reduce on DVE ----------
        for oc in range(nout):
            a0 = oc * JMG
            an = min(JMG, na - a0)
            b0 = na + a0
            bn = max(0, min(JMG, tpc - b0))
            ht = ht_tiles.pop(oc)
            l2_tiles.pop(oc)

            # psmg [P, 32, 32] = exactly 2 psum banks; 32-wide diff slots never
            # cross a bank boundary. Slots 0:15 = A diffs, 15:30 = B diffs,
            # slot 30 col i = A-tile-i invc, slot 31 col i = B-tile-i invc.
            psmg = psMg.tile([P, 2 * JMG + 2, D], F32, tag="psmg")
            for i in range(an):
                hta = ht[:K, i * P : (i + 1) * P]
                # gather(mu-eps) opens the group; negid accumulates -x
                nc.tensor.matmul(
                    psmg[:, i, :], hta, table[:K, :D], start=True, stop=False
                )
                nc.tensor.matmul(
                    psmg[:, i, :], negid, xe[:, a0 + i, :],
                    start=False, stop=True,
                )
                nc.tensor.matmul(
                    psmg[:, 2 * JMG, i : i + 1], hta, table[:K, D : D + 1],
                    start=True, stop=True,
                )
            for i in range(bn):
                htb = ht[K:, i * P : (i + 1) * P]
                s = JMG + i
                nc.tensor.matmul(
                    psmg[:, s, :], htb, table[K:, :D], start=True, stop=False
                )
                nc.tensor.matmul(
                    psmg[:, s, :], negid, xe[:, b0 + i, :],
                    start=False, stop=True,
                )
                nc.tensor.matmul(
                    psmg[:, 2 * JMG + 1, i : i + 1], htb, table[K:, D : D + 1],
                    start=True, stop=True,
                )

            # invc gather extraction (DVE: latency-critical for the psum ring)
            if an == JMG and bn == JMG:
                # one strided instr covers the A and B column ranges at once
                nc.vector.tensor_scalar_add(
                    bass.AP(
                        tensor=invc_all.tensor,
                        offset=invc_all.offset + a0,
                        ap=[list(invc_all.ap[0]), [na, 2], [1, JMG]],
                    ),
                    psmg[:, 2 * JMG :, :JMG],
                    0.0,
                )
            else:
                nc.vector.tensor_scalar_add(
                    invc_all[:, a0 : a0 + an], psmg[:, 2 * JMG, :an], 0.0
                )
                if bn > 0:
                    nc.vector.tensor_scalar_add(
                        invc_all[:, b0 : b0 + bn], psmg[:, 2 * JMG + 1, :bn], 0.0
                    )
            # square + halving-add reduction over D=32 (packed bf16 -> 2x DVE)
            sq = sqpool.tile([P, 2 * JMG, D], BF16, tag="sq")
            h16 = hvpool.tile([P, 2 * JMG, 16], BF16, tag="h16")
            h8 = hvpool.tile([P, 2 * JMG, 8], BF16, tag="h8")
            h4 = hvpool.tile([P, 2 * JMG, 4], BF16, tag="h4")
            h2t = hvpool.tile([P, 2 * JMG, 2], BF16, tag="h2t")
            if an == JMG and bn == JMG:
                ranges = [(0, 2 * JMG)]  # one combined pass over all 30 slots
            else:
                ranges = [(0, an)] + ([(JMG, bn)] if bn > 0 else [])
            for s0, n in ranges:
                if False:
                    # a few squares on DVE to keep Act off the critical path
                    nc.vector.tensor_mul(
                        sq[:, s0 : s0 + n, :],
                        psmg[:, s0 : s0 + n, :], psmg[:, s0 : s0 + n, :],
                    )
                else:
                    nc.scalar.activation(
                        out=sq[:, s0 : s0 + n, :], in_=psmg[:, s0 : s0 + n, :],
                        func=mybir.ActivationFunctionType.Square,
                    )
                nc.vector.tensor_add(
                    h16[:, s0 : s0 + n, :],
                    sq[:, s0 : s0 + n, 0:16], sq[:, s0 : s0 + n, 16:32],
                )
                nc.vector.tensor_add(
                    h8[:, s0 : s0 + n, :],
                    h16[:, s0 : s0 + n, 0:8], h16[:, s0 : s0 + n, 8:16],
                )
                nc.vector.tensor_add(
                    h4[:, s0 : s0 + n, :],
                    h8[:, s0 : s0 + n, 0:4], h8[:, s0 : s0 + n, 4:8],
                )
                nc.vector.tensor_add(
                    h2t[:, s0 : s0 + n, :],
                    h4[:, s0 : s0 + n, 0:2], h4[:, s0 : s0 + n, 2:4],
                )
            if an == JMG and bn == JMG:
                d2v = bass.AP(
                    tensor=d2all.tensor,
                    offset=d2all.offset + a0,
                    ap=[list(d2all.ap[0]), [na, 2], [1, JMG]],
                )
                hv0 = bass.AP(
                    tensor=h2t.tensor, offset=h2t.offset,
                    ap=[list(h2t.ap[0]), [2 * JMG, 2], [2, JMG]],
                )
                hv1 = bass.AP(
                    tensor=h2t.tensor, offset=h2t.offset + 1,
                    ap=[list(h2t.ap[0]), [2 * JMG, 2], [2, JMG]],
                )
                nc.vector.tensor_add(d2v, hv0, hv1)
            else:
                nc.vector.tensor_add(
                    d2all[:, a0 : a0 + an], h2t[:, :an, 0], h2t[:, :an, 1]
                )
                if bn > 0:
                    nc.vector.tensor_add(
                        d2all[:, b0 : b0 + bn],
                        h2t[:, JMG : JMG + bn, 0],
                        h2t[:, JMG : JMG + bn, 1],
                    )



        # ---------- inter + reg (tiny, replicated) ----------
        mup = wpool.tile([K, D], F32, tag="mup")
        nc.vector.tensor_scalar_add(mup, mu[:K, :], EPS)
        qsc = wpool.tile([K, D], F32, tag="qsc")
        nc.vector.tensor_mul(qsc, mu[:K, :], mu[:K, :])
        q = wpool.tile([K, 1], F32, tag="q")
        nc.vector.tensor_reduce(
            out=q, in_=qsc, axis=mybir.AxisListType.X, op=mybir.AluOpType.add
        )
        qpsc = wpool.tile([K, D], F32, tag="qpsc")
        nc.vector.tensor_mul(qpsc, mup, mup)
        qp = wpool.tile([K, 1], F32, tag="qp")
        nc.vector.tensor_reduce(
            out=qp, in_=qpsc, axis=mybir.AxisListType.X, op=mybir.AluOpType.add
        )
        # pd2[a,b] = qp_a - 2*mup_a.mu_b + q_b via one [64,64] matmul
        ab = wpool.tile([K, D + 2], F32, tag="ab")  # [-2*mup | qp | 1]
        nc.gpsimd.tensor_scalar_mul(ab[:, :D], mup, -2.0)
        nc.gpsimd.tensor_scalar_add(ab[:, D : D + 1], qp, 0.0)
        nc.vector.memset(ab[:, D + 1 : D + 2], 1.0)
        bb = wpool.tile([K, D + 2], F32, tag="bb")  # [mu | 1 | q]
        nc.gpsimd.tensor_scalar_add(bb[:, :D], mu[:K, :], 0.0)
        nc.vector.memset(bb[:, D : D + 1], 1.0)
        nc.gpsimd.tensor_scalar_add(bb[:, D + 1 : D + 2], q, 0.0)
        psT = psS.tile([D + 2, K], F32, tag="small")
        nc.tensor.transpose(psT, ab, id64)
        atp = wpool.tile([D + 2, K], F32, tag="atp")
        nc.scalar.copy(out=atp, in_=psT)
        psT2 = psS.tile([D + 2, K], F32, tag="small")
        nc.tensor.transpose(psT2, bb, id64)
        btp = wpool.tile([D + 2, K], F32, tag="btp")
        nc.scalar.copy(out=btp, in_=psT2)
        psPD = psS.tile([K, K], F32, tag="small")
        nc.tensor.matmul(psPD, atp, btp)
        pdc = wpool.tile([K, K], F32, tag="pdc")
        nc.vector.tensor_scalar_max(pdc, psPD, 0.0)
        pdist = wpool.tile([K, K], F32, tag="pdist")
        nc.scalar.activation(
            out=pdist, in_=pdc, func=mybir.ActivationFunctionType.Sqrt
        )
        hingeI = wpool.tile([K, K], F32, tag="hingeI")
        nc.scalar.activation(
            out=hingeI, in_=pdist, func=mybir.ActivationFunctionType.Relu,
            bias=float(INTER_MARGIN2), scale=-1.0,
        )
        hm = wpool.tile([K, K], F32, tag="hm")
        nc.vector.tensor_mul(hm, hingeI, eyeneg)
        hm2 = wpool.tile([K, K], F32, tag="hm2")
        nc.vector.tensor_mul(hm2, hm, hm)
        interp = wpool.tile([K, 1], F32, tag="interp")
        nc.vector.tensor_reduce(
            out=interp, in_=hm2, axis=mybir.AxisListType.X, op=mybir.AluOpType.add
        )
        sqp = wpool.tile([K, 1], F32, tag="sqp")
        nc.scalar.activation(
            out=sqp, in_=qp, func=mybir.ActivationFunctionType.Sqrt
        )
        cat2 = wpool.tile([K, 2], F32, tag="cat2")
        nc.gpsimd.tensor_scalar_add(cat2[:, 0:1], interp, 0.0)
        nc.gpsimd.tensor_scalar_add(cat2[:, 1:2], sqp, 0.0)
        psIR = psS.tile([1, 2], F32, tag="small")
        nc.tensor.matmul(psIR, ones64, cat2)
        ir = wpool.tile([1, 2], F32, tag="ir")  # [inter_sum, reg_sum]
        nc.scalar.copy(out=ir, in_=psIR)

        # ---------- intra finals (last segment + accumulation) ----------
        emit_finals_segment(seg_done[0], na, seg_done[1], tpc)
        ones128f = singles.tile([P, 1], F32)
        nc.vector.memset(ones128f, 1.0)
        psL = psS.tile([1, 1], F32, tag="small")
        nc.tensor.matmul(psL, racc_tiles[-1], ones128f)
        intra = wpool.tile([1, 1], F32, tag="intra")
        nc.scalar.copy(out=intra, in_=psL)
        nc.sync.dma_start(out=out_d[0:1], in_=intra[0:1, :])
        nc.sync.dma_start(out=out_d[1:3], in_=ir[0:1, :])

    nc.compile()
    return nc


_NC_CACHE = {}


def _get_program(tpc):
    if tpc not in _NC_CACHE:
        _NC_CACHE[tpc] = build_program(tpc)
    return _NC_CACHE[tpc]


def kernel(features, labels, num_clusters):
    features = np.asarray(features)
    labels = np.asarray(labels)
    n_total = features.shape[0]
    n_core = n_total // N_CORES
    tpc = math.ceil(n_core / P)
    nc = _get_program(tpc)
    in_maps = _host_prep(features, labels, tpc)
    res = run_bass_kernel_spmd(nc, in_maps, list(range(N_CORES)))
    intra_sum = sum(float(res.results[c]["out"][0]) for c in range(N_CORES))
    inter_sum = float(res.results[0]["out"][1])
    reg_sum = float(res.results[0]["out"][2])
    total = (
        intra_sum / K
        + inter_sum / (K * (K - 1))
        + 0.001 * reg_sum / K
    )
    return np.float32(total)


# revision 57
# speedup vs baseline: 1.0016x; 1.0016x over previous
# kernel.py — DiscriminativeLoss on 8 TRN2 NeuronCores (Bass/Tile, SPMD).
#
# Math (matches reference):
#   counts_k = #{i: l_i = k};  S_k = sum_{i in k} x_i;  mu_k = S_k / max(c_k, 1)
#   intra = (1/K) * sum_i invc_{l_i} * relu(||x_i - mu_{l_i} + eps|| - 1.5)^2
#   inter = sum_{a != b} relu(1 - ||(mu_a + eps) - mu_b||)^2 / (K*(K-1))
#   reg   = (1/K) * sum_k ||mu_k + eps||
#   total = intra + inter + 0.001 * reg
#
# Device strategy (per core, data-parallel over points; point i = p*tpc + j
# lives at [p, j]):
#   pass 1: one-hot H2 [128, 64, J1] built per chunk via a single DVE
#     tensor_tensor is_equal against a materialized replicated iota (all
#     operands packed 2-byte -> 2x DVE mode); per-tile PE matmuls
#     lhsT=H2[:, :, j] [128, 64] x rhs=xe[:, j, :] accumulate S^T [64, 32]
#     and (x ones-rhs) counts [64, 1] (N<=32 -> cheap).
#   AllReduce [64, 33] across 8 cores (28us fixed cost; overlapped with
#     pass-2 one-hot prebuilds, label-row DMA prefetch, and PE keep-warm
#     dummy matmuls that hold the tensor engine's p-state at full clock).
#   stats: invc = 1/max(c,1), mu = S*invc; the reduced block is DMA'd back
#     duplicated into both partition halves so table [128, 33] =
#     [mu-eps | invc] (rows 64:128 serve the B-half pairing) needs no
#     replication hop.
#   pass 2, per outer chunk of 15 A-tiles + 15 B-tiles: transposed one-hot
#     ht [128, 15*128] built at 4x DVE (TensorScalarPtr is_equal vs the
#     per-partition iota; Pool builds the later chunks) from a
#     broadcast-DMA'd label row; per tile, PE accumulates
#     psum[:, slot, :] = gather(mu-eps) - x  (the diff formed entirely on
#     PE via a -Identity matmul) and a 1-col gather of invc into slots
#     30/31 of the bank-aligned [128, 32, 32] psum tile; Act squares the
#     PSUM diff to bf16; DVE reduces via log2 halving adds (packed bf16 ->
#     2x mode) and extracts invc.
#   finals: dist = sqrt(d2); hinge = relu(dist-1.5); intra partial =
#     sum hinge^2 * invc via muls + row reduce (chained across two column
#     segments) + ones-matmul partition reduce.
#   inter/reg (KxK) replicated on every core from the reduced stats.
#   (GPSIMD constraints honored for real HW: no PSUM access, no is_equal
#   TensorTensor; all one-hot compares on Pool use TensorScalarPtr.)
import math
import numpy as np
from contextlib import ExitStack

import concourse.bass as bass
import concourse.bacc as bacc
import concourse.tile as tile
import concourse.mybir as mybir
from concourse.bass_utils import run_bass_kernel_spmd

F32 = mybir.dt.float32
BF16 = mybir.dt.bfloat16
I16 = mybir.dt.int16

N_CORES = 8
K = 64
D = 32
P = 128
EPS = 1e-8
PAD_LABEL = 512  # never matches any one-hot row (0..127); exact in bf16

INTRA_MARGIN = 1.5
INTER_MARGIN2 = 1.0  # 2 * 0.5

J1 = 30       # pass-1 tiles per one-hot chunk
JMG = 15      # pass-2 A-tiles (and B-tiles) per outer chunk
PREBUILD = 21   # pass-2 ht chunks emitted before the collective section
L2_BUFS = 5
HT_BUFS = 20


def _host_prep(features, labels, tpc):
    """Shard + relayout on host. Returns per-core input dicts."""
    n_total = features.shape[0]
    n_core = n_total // N_CORES
    n_pad = P * tpc
    import ml_dtypes

    na = (tpc + 1) // 2
    nout = math.ceil(na / JMG)
    iotacol = np.arange(P, dtype=np.float32).reshape(P, 1)
    negid = (-np.eye(P)).astype(ml_dtypes.bfloat16)
    id64 = np.eye(K, dtype=np.float32)
    eyeneg = (1.0 - np.eye(K, dtype=np.float32)).astype(ml_dtypes.bfloat16)

    in_maps = []
    for c in range(N_CORES):
        f = np.asarray(features[c * n_core : (c + 1) * n_core], dtype=np.float32)
        l = np.asarray(labels[c * n_core : (c + 1) * n_core], dtype=np.int64)
        if n_pad > n_core:
            f = np.concatenate([f, np.zeros((n_pad - n_core, D), np.float32)], axis=0)
            l = np.concatenate([l, np.full((n_pad - n_core,), PAD_LABEL, np.int64)])
        # xe: [P, tpc, 32] bf16 (counts come from separate ones-rhs matmuls)
        xe = f.reshape(P, tpc, D).astype(ml_dtypes.bfloat16)
        # p-major labels (pass-1 one-hot): [P, tpc] bf16 (values exact)
        lpm = l.reshape(P, tpc).astype(ml_dtypes.bfloat16)
        # tile-major labels for pass 2: ltm [nout, 2, JMG*P] int16,
        # [oc, 0] = A-tile labels, [oc, 1] = B-tile labels + 64.
        ltm_full = l.reshape(P, tpc).T.astype(np.float32)  # [tpc, P]
        ltm = np.full((nout, 2, JMG * P), PAD_LABEL, ml_dtypes.bfloat16)
        for oc in range(nout):
            a0 = oc * JMG
            an = min(JMG, na - a0)
            ltm[oc, 0, : an * P] = ltm_full[a0 : a0 + an].ravel()
            b0 = na + a0
            bn = max(0, min(JMG, tpc - b0))
            if bn > 0:
                ltm[oc, 1, : bn * P] = ltm_full[b0 : b0 + bn].ravel() + 64
        in_maps.append(
            {
                "xe": np.ascontiguousarray(xe),
                "lpm": np.ascontiguousarray(lpm),
                "ltm": np.ascontiguousarray(ltm),
                "iotacol": iotacol,
                "negid": negid,
                "id64": id64,
                "eyeneg": eyeneg,
            }
        )
    return in_maps


def build_program(tpc):
    """Build the SPMD Bass program. tpc = tiles per core (cols per partition)."""
    nc = bacc.Bacc(
        "TRN2", target_bir_lowering=False, debug=False, num_devices=N_CORES
    )
    core_ids = list(range(N_CORES))

    na = (tpc + 1) // 2
    nout = math.ceil(na / JMG)
    n_chunks1 = math.ceil(tpc / J1)

    xe_d = nc.dram_tensor("xe", [P, tpc, D], BF16, kind="ExternalInput").ap()
    lpm_d = nc.dram_tensor("lpm", [P, tpc], BF16, kind="ExternalInput").ap()
    ltm_d = nc.dram_tensor("ltm", [nout, 2, JMG * P], BF16, kind="ExternalInput").ap()
    iotacol_d = nc.dram_tensor("iotacol", [P, 1], F32, kind="ExternalInput").ap()
    negid_d = nc.dram_tensor("negid", [P, P], BF16, kind="ExternalInput").ap()
    id64_d = nc.dram_tensor("id64", [K, K], F32, kind="ExternalInput").ap()
    eyeneg_d = nc.dram_tensor("eyeneg", [K, K], BF16, kind="ExternalInput").ap()
    out_d = nc.dram_tensor("out", [3], F32, kind="ExternalOutput").ap()

    with tile.TileContext(nc, num_cores=N_CORES) as tc, ExitStack() as ctx:
        singles = ctx.enter_context(tc.tile_pool(name="singles", bufs=1))
        xpool = ctx.enter_context(tc.tile_pool(name="xpool", bufs=1))
        hpool = ctx.enter_context(tc.tile_pool(name="hpool", bufs=4))
        l2pool = ctx.enter_context(tc.tile_pool(name="l2pool", bufs=L2_BUFS))
        htpool = ctx.enter_context(tc.tile_pool(name="htpool", bufs=HT_BUFS))
        sqpool = ctx.enter_context(tc.tile_pool(name="sqpool", bufs=3))
        hvpool = ctx.enter_context(tc.tile_pool(name="hvpool", bufs=3))
        wpool = ctx.enter_context(tc.tile_pool(name="wpool", bufs=2))
        psA = ctx.enter_context(tc.tile_pool(name="psA", bufs=1, space="PSUM"))
        psMg = ctx.enter_context(tc.tile_pool(name="psMg", bufs=3, space="PSUM"))
        psS = ctx.enter_context(tc.tile_pool(name="psS", bufs=1, space="PSUM"))
        dram = ctx.enter_context(tc.tile_pool(name="dram", bufs=2, space="DRAM"))

        # ---------- constants (critical first: pass-1 inputs) ----------
        lpm = singles.tile([P, tpc], BF16)
        # first pass-1 chunk's labels land first (tiny strided DMA) so the
        # H2 stream starts ~0.6us earlier; the rest follows immediately
        j1c = min(J1, tpc)
        nc.sync.dma_start(out=lpm[:, :j1c], in_=lpm_d[:, :j1c])
        if tpc > j1c:
            nc.sync.dma_start(out=lpm[:, j1c:], in_=lpm_d[:, j1c:])
        # replicated iota built on-chip by the (otherwise idle) Pool engine:
        # keeps the pass-1 critical path down to the single small lpm DMA
        iota_rep = singles.tile([P, K, J1], BF16)
        nc.gpsimd.iota(
            iota_rep, pattern=[[1, K], [0, J1]], base=0,
            channel_multiplier=0, allow_small_or_imprecise_dtypes=True,
        )
        margneg = singles.tile([P, 1], F32)
        nc.vector.memset(margneg, -float(INTRA_MARGIN))
        ones64 = singles.tile([K, 1], F32)
        nc.vector.memset(ones64, 1.0)
        # prewarm the Act function table with Sqrt: narrows the possible
        # table sets to one containing sqrt+square+relu+copy, so the single
        # 1.3us load happens here, off-critical, and never again
        actwarm = singles.tile([1, 1], F32)
        nc.scalar.activation(
            out=actwarm, in_=ones64[0:1, :],
            func=mybir.ActivationFunctionType.Sqrt,
        )
        xe = xpool.tile([P, tpc, D], BF16)
        ones128 = singles.tile([P, 1], BF16)
        nc.vector.memset(ones128, 1.0)

        d2all = singles.tile([P, tpc], BF16)
        invc_all = singles.tile([P, tpc], BF16)

        # ---------- pass 1: segment sums S^T [64, 32] + counts [64, 1] ----------
        psumS = psA.tile([K, D], F32)
        psumC = psS.tile([K, 1], F32, tag="small")
        t_done = 0
        for c in range(n_chunks1):
            j0 = c * J1
            jn = min(J1, tpc - j0)
            nc.sync.dma_start(
                out=xe[:, j0 : j0 + jn, :], in_=xe_d[:, j0 : j0 + jn, :]
            )
            h2 = hpool.tile([P, K, J1], BF16, tag="h2")
            nc.vector.tensor_tensor(
                h2[:, :, :jn],
                lpm[:, None, j0 : j0 + jn].to_broadcast((P, K, jn)),
                iota_rep[:, :, :jn],
                mybir.AluOpType.is_equal,
            )
            for j in range(jn):
                nc.tensor.matmul(
                    psumS,
                    h2[:, :, j],
                    xe[:, j0 + j, :],
                    start=(t_done == 0),
                    stop=(t_done == tpc - 1),
                )
                nc.tensor.matmul(
                    psumC,
                    h2[:, :, j],
                    ones128,
                    start=(t_done == 0),
                    stop=(t_done == tpc - 1),
                )
                t_done += 1

        # ---------- remaining constants (needed only from pass 2 on) ----------
        iotacol = singles.tile([P, 1], F32)
        nc.sync.dma_start(out=iotacol, in_=iotacol_d)
        negid = singles.tile([P, P], BF16)
        nc.sync.dma_start(out=negid, in_=negid_d)
        id64 = singles.tile([K, K], F32)
        nc.sync.dma_start(out=id64, in_=id64_d)
        eyeneg = singles.tile([K, K], BF16)
        nc.sync.dma_start(out=eyeneg, in_=eyeneg_d)

        # ---------- pass-2 prep: prebuild label rows + transposed one-hots ----
        # (no dependency on the collective -> fills the AllReduce window)
        l2_tiles = {}
        ht_tiles = {}

        def emit_l2_ht(oc, eng=nc.vector):
            src = ltm_d[oc]
            l2 = l2pool.tile([P, JMG * P], BF16, tag="l2")
            nc.sync.dma_start(
                out=l2,
                in_=bass.AP(
                    tensor=src.tensor,
                    offset=src.offset,
                    ap=[[JMG * P, 2], [0, K]] + [[1, JMG * P]],
                ),
            )
            ht = htpool.tile([P, JMG * P], BF16, tag="ht")
            eng.tensor_single_scalar(
                ht, l2, iotacol, mybir.AluOpType.is_equal
            )
            l2_tiles[oc] = l2
            ht_tiles[oc] = ht

        # ---------- early pass-2 one-hot prebuilds (before the stats block
        # so the l2-gated tail can't delay the table computation) ----------
        for oc in range(min(15, nout)):
            emit_l2_ht(oc)

        # ---------- AllReduce the [64, 33] stats ----------
        sg_local = wpool.tile([K, D + 1], F32, tag="sg")
        nc.scalar.copy(out=sg_local[:, :D], in_=psumS)
        nc.scalar.copy(out=sg_local[:, D : D + 1], in_=psumC)
        cc_in = dram.tile([K, D + 1], F32)
        cc_out = dram.tile([K, D + 1], F32)
        nc.gpsimd.dma_start(out=cc_in, in_=sg_local)
        nc.gpsimd.collective_compute(
            "AllReduce",
            mybir.AluOpType.add,
            replica_groups=[core_ids],
            ins=[cc_in.opt()],
            outs=[cc_out.opt()],
        )

        # ---------- PE keep-warm during the collective ----------
        # the tensor engine p-state drops after ~idle; feed it junk matmuls
        # (into the recycled small-psum slot) so pass-2 gathers start at
        # full clock. No data deps; they fill the AllReduce window.
        junkps = psS.tile([K, 512], F32, tag="small")
        warm_tiles = min(16, tpc)
        for w in range(150):
            nc.tensor.matmul(
                junkps[:, : warm_tiles * D], negid[:, :K],
                xe[:, 0:warm_tiles, :], start=True, stop=True,
            )

        # ---------- remaining prebuilds + all Pool-side one-hot builds ------
        # (DVE 16..20 after the stats so they can't delay the table; Pool
        # takes the rest, always ahead of consumption via the ht ring)
        for oc in range(15, nout):
            emit_l2_ht(oc, nc.gpsimd)


        # per-point finals tiles + helper (emitted in segments so most of
        # the sqrt/relu/mul/accumulate work overlaps pass 2). The running
        # per-partition intra sum is chained through tensor_tensor_reduce's
        # accumulator seed.
        dist = singles.tile([P, tpc], BF16)
        hinge = singles.tile([P, tpc], BF16)
        hsq = dist  # dist is dead after the relu; reuse its storage
        hv = singles.tile([P, tpc], BF16)
        seg_done = (0, na)
        racc_tiles = []

        def emit_finals_segment(ca0, ca1, cb0, cb1):
            for c0, c1 in ((ca0, ca1), (cb0, cb1)):
                if c1 <= c0:
                    continue
                nc.scalar.activation(
                    out=dist[:, c0:c1], in_=d2all[:, c0:c1],
                    func=mybir.ActivationFunctionType.Sqrt,
                )
                nc.scalar.activation(
                    out=hinge[:, c0:c1], in_=dist[:, c0:c1],
                    func=mybir.ActivationFunctionType.Relu, bias=margneg,
                )
                nc.vector.tensor_mul(
                    hsq[:, c0:c1], hinge[:, c0:c1], hinge[:, c0:c1]
                )
                racc = singles.tile([P, 1], F32, tag=f"racc{len(racc_tiles)}")
                nc.vector.tensor_mul(hv[:, c0:c1], hsq[:, c0:c1], invc_all[:, c0:c1])
                nc.vector.tensor_reduce(
                    out=racc, in_=hv[:, c0:c1], axis=mybir.AxisListType.X,
                    op=mybir.AluOpType.add,
                )
                if racc_tiles:
                    nc.vector.tensor_add(racc, racc, racc_tiles[-1])
                racc_tiles.append(racc)

        # fetch the reduced stats duplicated into both partition halves so
        # the whole [128, 33] table is computed in one go (no replication
        # hop). high_priority so the scheduler doesn't slot a 1.6us Pool
        # one-hot build ahead of it.
        sg = singles.tile([P, D + 1], F32)
        nc.scalar.dma_start(
            out=sg,
            in_=bass.AP(
                tensor=cc_out.tensor,
                offset=cc_out.offset,
                ap=[[0, 2], [D + 1, K], [1, D + 1]],
            ),
        )

        # ---------- critical stats: table = [mu - eps | invc] ----------
        # high_priority: the scheduler must not queue l2-gated one-hot
        # builds ahead of these on DVE -- the whole of pass 2 waits on table
        with tc.high_priority():
            safec = singles.tile([P, 1], F32)
            nc.vector.tensor_scalar_max(safec, sg[:, D : D + 1], 1.0)
            invc = singles.tile([P, 1], F32)
            nc.vector.reciprocal(invc, safec)
            mu = singles.tile([P, D], F32)
            nc.vector.tensor_mul(mu, sg[:, :D], invc.to_broadcast((P, D)))
            table = singles.tile([P, D + 1], BF16)
            nc.vector.tensor_scalar_add(table[:, :D], mu, -EPS)
            nc.vector.tensor_scalar_add(table[:, D : D + 1], invc, 0.0)

        # ---------- pass 2: diff via PE, square on Act, reduce on DVE ----------
        for oc in range(nout):
            a0 = oc * JMG
            an = min(JMG, na - a0)
            b0 = na + a0
            bn = max(0, min(JMG, tpc - b0))
            ht = ht_tiles.pop(oc)
            l2_tiles.pop(oc)

            # psmg [P, 32, 32] = exactly 2 psum banks; 32-wide diff slots never
            # cross a bank boundary. Slots 0:15 = A diffs, 15:30 = B diffs,
            # slot 30 col i = A-tile-i invc, slot 31 col i = B-tile-i invc.
            psmg = psMg.tile([P, 2 * JMG + 2, D], F32, tag="psmg")
            for i in range(an):
                hta = ht[:K, i * P : (i + 1) * P]
                # gather(mu-eps) opens the group; negid accumulates -x
                nc.tensor.matmul(
                    psmg[:, i, :], hta, table[:K, :D], start=True, stop=False
                )
                nc.tensor.matmul(
                    psmg[:, i, :], negid, xe[:, a0 + i, :],
                    start=False, stop=True,
                )
                nc.tensor.matmul(
                    psmg[:, 2 * JMG, i : i + 1], hta, table[:K, D : D + 1],
                    start=True, stop=True,
                )
            for i in range(bn):
                htb = ht[K:, i * P : (i + 1) * P]
                s = JMG + i
                nc.tensor.matmul(
                    psmg[:, s, :], htb, table[K:, :D], start=True, stop=False
                )
                nc.tensor.matmul(
                    psmg[:, s, :], negid, xe[:, b0 + i, :],
                    start=False, stop=True,
                )
                nc.tensor.matmul(
                    psmg[:, 2 * JMG + 1, i : i + 1], htb, table[K:, D : D + 1],
                    start=True, stop=True,
                )

            # invc gather extraction (DVE: latency-critical for the psum ring)
            if an == JMG and bn == JMG:
                # one strided instr covers the A and B column ranges at once
                nc.vector.tensor_scalar_add(
                    bass.AP(
                        tensor=invc_all.tensor,
                        offset=invc_all.offset + a0,
                        ap=[list(invc_all.ap[0]), [na, 2], [1, JMG]],
                    ),
                    psmg[:, 2 * JMG :, :JMG],
                    0.0,
                )
            else:
                nc.vector.tensor_scalar_add(
                    invc_all[:, a0 : a0 + an], psmg[:, 2 * JMG, :an], 0.0
                )
                if bn > 0:
                    nc.vector.tensor_scalar_add(
                        invc_all[:, b0 : b0 + bn], psmg[:, 2 * JMG + 1, :bn], 0.0
                    )
            # square + halving-add reduction over D=32 (packed bf16 -> 2x DVE)
            sq = sqpool.tile([P, 2 * JMG, D], BF16, tag="sq")
            h16 = hvpool.tile([P, 2 * JMG, 16], BF16, tag="h16")
            h8 = hvpool.tile([P, 2 * JMG, 8], BF16, tag="h8")
            h4 = hvpool.tile([P, 2 * JMG, 4], BF16, tag="h4")
            h2t = hvpool.tile([P, 2 * JMG, 2], BF16, tag="h2t")
            if an == JMG and bn == JMG:
                ranges = [(0, 2 * JMG)]  # one combined pass over all 30 slots
            else:
                ranges = [(0, an)] + ([(JMG, bn)] if bn > 0 else [])
            for s0, n in ranges:
                if False:
                    # a few squares on DVE to keep Act off the critical path
                    nc.vector.tensor_mul(
                        sq[:, s0 : s0 + n, :],
                        psmg[:, s0 : s0 + n, :], psmg[:, s0 : s0 + n, :],
                    )
                else:
                    nc.scalar.activation(
                        out=sq[:, s0 : s0 + n, :], in_=psmg[:, s0 : s0 + n, :],
                        func=mybir.ActivationFunctionType.Square,
                    )
                nc.vector.tensor_add(
                    h16[:, s0 : s0 + n, :],
                    sq[:, s0 : s0 + n, 0:16], sq[:, s0 : s0 + n, 16:32],
                )
                nc.vector.tensor_add(
                    h8[:, s0 : s0 + n, :],
                    h16[:, s0 : s0 + n, 0:8], h16[:, s0 : s0 + n, 8:16],
                )
                nc.vector.tensor_add(
                    h4[:, s0 : s0 + n, :],
                    h8[:, s0 : s0 + n, 0:4], h8[:, s0 : s0 + n, 4:8],
                )
                nc.vector.tensor_add(
                    h2t[:, s0 : s0 + n, :],
                    h4[:, s0 : s0 + n, 0:2], h4[:, s0 : s0 + n, 2:4],
                )
            if an == JMG and bn == JMG:
                d2v = bass.AP(
                    tensor=d2all.tensor,
                    offset=d2all.offset + a0,
                    ap=[list(d2all.ap[0]), [na, 2], [1, JMG]],
                )
                hv0 = bass.AP(
                    tensor=h2t.tensor, offset=h2t.offset,
                    ap=[list(h2t.ap[0]), [2 * JMG, 2], [2, JMG]],
                )
                hv1 = bass.AP(
                    tensor=h2t.tensor, offset=h2t.offset + 1,
                    ap=[list(h2t.ap[0]), [2 * JMG, 2], [2, JMG]],
                )
                nc.vector.tensor_add(d2v, hv0, hv1)
            else:
                nc.vector.tensor_add(
                    d2all[:, a0 : a0 + an], h2t[:, :an, 0], h2t[:, :an, 1]
                )
                if bn > 0:
                    nc.vector.tensor_add(
                        d2all[:, b0 : b0 + bn],
                        h2t[:, JMG : JMG + bn, 0],
                        h2t[:, JMG : JMG + bn, 1],
                    )



        # ---------- inter + reg (tiny, replicated) ----------
        mup = wpool.tile([K, D], F32, tag="mup")
        nc.vector.tensor_scalar_add(mup, mu[:K, :], EPS)
        qsc = wpool.tile([K, D], F32, tag="qsc")
        nc.vector.tensor_mul(qsc, mu[:K, :], mu[:K, :])
        q = wpool.tile([K, 1], F32, tag="q")
        nc.vector.tensor_reduce(
            out=q, in_=qsc, axis=mybir.AxisListType.X, op=mybir.AluOpType.add
        )
        qpsc = wpool.tile([K, D], F32, tag="qpsc")
        nc.vector.tensor_mul(qpsc, mup, mup)
        qp = wpool.tile([K, 1], F32, tag="qp")
        nc.vector.tensor_reduce(
            out=qp, in_=qpsc, axis=mybir.AxisListType.X, op=mybir.AluOpType.add
        )
        # pd2[a,b] = qp_a - 2*mup_a.mu_b + q_b via one [64,64] matmul
        ab = wpool.tile([K, D + 2], F32, tag="ab")  # [-2*mup | qp | 1]
        nc.gpsimd.tensor_scalar_mul(ab[:, :D], mup, -2.0)
        nc.gpsimd.tensor_scalar_add(ab[:, D : D + 1], qp, 0.0)
        nc.vector.memset(ab[:, D + 1 : D + 2], 1.0)
        bb = wpool.tile([K, D + 2], F32, tag="bb")  # [mu | 1 | q]
        nc.gpsimd.tensor_scalar_add(bb[:, :D], mu[:K, :], 0.0)
        nc.vector.memset(bb[:, D : D + 1], 1.0)
        nc.gpsimd.tensor_scalar_add(bb[:, D + 1 : D + 2], q, 0.0)
        psT = psS.tile([D + 2, K], F32, tag="small")
        nc.tensor.transpose(psT, ab, id64)
        atp = wpool.tile([D + 2, K], F32, tag="atp")
        nc.scalar.copy(out=atp, in_=psT)
        psT2 = psS.tile([D + 2, K], F32, tag="small")
        nc.tensor.transpose(psT2, bb, id64)
        btp = wpool.tile([D + 2, K], F32, tag="btp")
        nc.scalar.copy(out=btp, in_=psT2)
        psPD = psS.tile([K, K], F32, tag="small")
        nc.tensor.matmul(psPD, atp, btp)
        pdc = wpool.tile([K, K], F32, tag="pdc")
        nc.vector.tensor_scalar_max(pdc, psPD, 0.0)
        pdist = wpool.tile([K, K], F32, tag="pdist")
        nc.scalar.activation(
            out=pdist, in_=pdc, func=mybir.ActivationFunctionType.Sqrt
        )
        hingeI = wpool.tile([K, K], F32, tag="hingeI")
        nc.scalar.activation(
            out=hingeI, in_=pdist, func=mybir.ActivationFunctionType.Relu,
            bias=float(INTER_MARGIN2), scale=-1.0,
        )
        hm = wpool.tile([K, K], F32, tag="hm")
        nc.vector.tensor_mul(hm, hingeI, eyeneg)
        hm2 = wpool.tile([K, K], F32, tag="hm2")
        nc.vector.tensor_mul(hm2, hm, hm)
        interp = wpool.tile([K, 1], F32, tag="interp")
        nc.vector.tensor_reduce(
            out=interp, in_=hm2, axis=mybir.AxisListType.X, op=mybir.AluOpType.add
        )
        sqp = wpool.tile([K, 1], F32, tag="sqp")
        nc.scalar.activation(
            out=sqp, in_=qp, func=mybir.ActivationFunctionType.Sqrt
        )
        cat2 = wpool.tile([K, 2], F32, tag="cat2")
        nc.gpsimd.tensor_scalar_add(cat2[:, 0:1], interp, 0.0)
        nc.gpsimd.tensor_scalar_add(cat2[:, 1:2], sqp, 0.0)
        psIR = psS.tile([1, 2], F32, tag="small")
        nc.tensor.matmul(psIR, ones64, cat2)
        ir = wpool.tile([1, 2], F32, tag="ir")  # [inter_sum, reg_sum]
        nc.scalar.copy(out=ir, in_=psIR)

        # ---------- intra finals (last segment + accumulation) ----------
        emit_finals_segment(seg_done[0], na, seg_done[1], tpc)
        ones128f = singles.tile([P, 1], F32)
        nc.vector.memset(ones128f, 1.0)
        psL = psS.tile([1, 1], F32, tag="small")
        nc.tensor.matmul(psL, racc_tiles[-1], ones128f)
        intra = wpool.tile([1, 1], F32, tag="intra")
        nc.scalar.copy(out=intra, in_=psL)
        nc.sync.dma_start(out=out_d[0:1], in_=intra[0:1, :])
        nc.sync.dma_start(out=out_d[1:3], in_=ir[0:1, :])

    nc.compile()
    return nc


_NC_CACHE = {}


def _get_program(tpc):
    if tpc not in _NC_CACHE:
        _NC_CACHE[tpc] = build_program(tpc)
    return _NC_CACHE[tpc]


def kernel(features, labels, num_clusters):
    features = np.asarray(features)
    labels = np.asarray(labels)
    n_total = features.shape[0]
    n_core = n_total // N_CORES
    tpc = math.ceil(n_core / P)
    nc = _get_program(tpc)
    in_maps = _host_prep(features, labels, tpc)
    res = run_bass_kernel_spmd(nc, in_maps, list(range(N_CORES)))
    intra_sum = sum(float(res.results[c]["out"][0]) for c in range(N_CORES))
    inter_sum = float(res.results[0]["out"][1])
    reg_sum = float(res.results[0]["out"][2])
    total = (
        intra_sum / K
        + inter_sum / (K * (K - 1))
        + 0.001 * reg_sum / K
    )
    return np.float32(total)
